# revision 45
# baseline (speedup 1.0000x reference)
"""Self-contained Trainium2 Bass kernel for nn_GCNMagnetModel (3-layer GCN,
N=50000 nodes, E=600000 edges, H=128, 64 graphs, 8 NeuronCores, SPMD 1 NEFF).

Sharding: nodes/edges sharded across 8 cores by graph id (graphs 8k..8k+7 ->
core k; graph-block-aligned node layout so both pools are core-local).

Dataflow (v2):
- Host computes all index layout AND integer degree counts (bincount); the
  device does rsqrt and every other FP op on tensor values.
- Layer 1 is rank-2: agg((x@W1)*dinv) == agg(x*dinv) @ W1, so layer-1 message
  passing gathers 2-wide u-rows (4B) instead of 128-wide table rows; the
  layer-1 table build and its AllGather disappear.
- Layers 2/3 gather bf16 table rows of t1 = (h@W)*dinv_src from an
  AllGather'd table; per dst-block-of-128 segment-sum via one-hot matmuls.
- The GCN self-loop term is one identity matmul from the SBUF-resident
  sbuild (t1) tile per block - no separate bstar machinery.
- Layers 1/2 aggregate in TRANSPOSED orientation (aggT[H,d] with the gathered
  chunk as the stationary operand) so h feeds the next layer's prepare matmul
  with no PE transpose; prepare is fused into the same block iteration, so no
  resident hT buffer exists. Layer 3 aggregates node-major for pooling.
- Per-(block,half) chunk counts are the max over the 8 cores (SPMD shapes)
  instead of a global max, roughly halving gather volume and one-hot builds.
- Each table AllGather is split into two contiguous half-shard collectives on
  SEPARATE DRAM tensors (half-major row numbering: region0 = all cores'
  blocks < NBLK/2, region1 = rest; the region boundary doubles as the int16
  A/B gather split). The first half-AG issues mid-loop so its transfer
  overlaps the remaining blocks' compute, and the next layer's A-half
  gathers depend only on it - the second half-AG overlaps their gathers.

kernel(**inputs) -> [64, 41] float32.
"""
import numpy as np
import ml_dtypes
from contextlib import ExitStack

import concourse.tile as tile
import concourse.mybir as mybir
from concourse import bacc
from concourse import library_config
from concourse.bass_utils import run_bass_kernel_spmd

NCORE = 8
P = 128
GPC = 8  # graphs per core
H = 128
OC = 41

F32 = mybir.dt.float32
BF16 = mybir.dt.bfloat16
I16 = mybir.dt.int16
AF = mybir.ActivationFunctionType
OP = mybir.AluOpType


def wrap16(v):  # [n] -> [128, n/16]: idx[i%16, i//16] tiled 8x
    a = v.reshape(-1, 16).T
    return np.tile(a, (8, 1)).copy()


def prep(x, edge_index, batch, n_graphs=64):
    N = x.shape[0]
    x = np.asarray(x, np.float32)
    batch = np.asarray(batch)
    src_g, dst_g = np.asarray(edge_index[0]), np.asarray(edge_index[1])
    E = src_g.shape[0]

    gstart = np.searchsorted(batch, np.arange(n_graphs), side="left")
    gend = np.searchsorted(batch, np.arange(n_graphs), side="right")
    gsz = gend - gstart

    gblk = np.maximum((gsz + P - 1) // P, 1)
    nblk_core = [int(gblk[k * GPC:(k + 1) * GPC].sum()) for k in range(NCORE)]
    NBLK = max(nblk_core)
    NMAXP = NBLK * P

    loc_base = np.zeros(n_graphs, np.int64)
    for g in range(n_graphs):
        if g % GPC == 0:
            loc_base[g] = 0
        else:
            loc_base[g] = loc_base[g - 1] + gblk[g - 1] * P
    node_core = batch // GPC
    node_loc = loc_base[batch] + (np.arange(N) - gstart[batch])
    # half-major rows: region0 = all cores' blocks < NB2 (k-major), region1 =
    # the rest; both regions contiguous so each half-shard AllGather is a
    # contiguous ins/outs pair, and region0 boundary doubles as the int16
    # A/B gather-table split.
    NB2 = NBLK // 2
    R0 = NCORE * NB2 * P
    node_b = node_loc // P
    node_p = node_loc % P
    node_row = np.where(
        node_b < NB2,
        node_core * (NB2 * P) + node_b * P + node_p,
        R0 + node_core * ((NBLK - NB2) * P) + (node_b - NB2) * P + node_p)

    HALF = R0
    assert HALF < 32768 and NCORE * NMAXP - HALF < 32768, f"HALF={HALF}"

    # host degree counts (integer index work); +1 self loop
    deg = np.bincount(dst_g, minlength=N).astype(np.float32) + 1.0

    # per-core padded layouts
    # deg_pm[k][p, b] ; degrow[k][loc] ; x node-major interleaved
    deg_pm = np.ones((NCORE, P, NBLK), np.float32)
    degrow = np.ones((NCORE, NMAXP), np.float32)
    xnm2 = np.zeros((NCORE, P, NBLK * 2), np.float32)
    pidx = (node_loc % P).astype(np.int64)
    bidx = (node_loc // P).astype(np.int64)
    deg_pm[node_core, pidx, bidx] = deg
    degrow[node_core, node_loc] = deg
    xnm2[node_core, pidx, bidx * 2] = x[:, 0]
    xnm2[node_core, pidx, bidx * 2 + 1] = x[:, 1]

    # edges -> (core, blk, half); chunk counts = per-(blk,half) max over cores
    e_core = node_core[dst_g]
    e_dstloc = node_loc[dst_g]
    e_blk = e_dstloc // P
    e_dl = (e_dstloc % P).astype(np.float32)
    e_row = node_row[src_g]
    e_half = (e_row >= HALF).astype(np.int64)

    cnts = np.zeros((NCORE, NBLK, 2), np.int64)
    np.add.at(cnts, (e_core, e_blk, e_half), 1)
    cp = (cnts.max(axis=0) + P - 1) // P          # [NBLK, 2] chunks
    cpA, cpB = cp[:, 0], cp[:, 1]
    offA = np.r_[0, np.cumsum(cpA)]               # [NBLK+1]
    offB = np.r_[0, np.cumsum(cpB)]
    NCHA, NCHB = int(offA[-1]), int(offB[-1])

    # slot assignment: sort edges by (core, half, blk), fill runs
    order = np.lexsort((e_blk, e_half, e_core))
    so_core, so_blk, so_half = e_core[order], e_blk[order], e_half[order]
    so_row, so_dl = e_row[order], e_dl[order]
    key = (so_core * 2 + so_half) * NBLK + so_blk
    runstart = np.r_[0, np.flatnonzero(np.diff(key)) + 1]
    runid = np.zeros(E, np.int64)
    runid[runstart[1:]] = 1
    runid = np.cumsum(runid)
    pos_in_run = np.arange(E) - runstart[runid]

    NCH = NCHA + NCHB
    idxA = np.zeros((NCORE, NCHA * P), np.int16)
    idxB = np.zeros((NCORE, NCHB * P), np.int16)
    dlA = np.full((NCORE, NCHA * P), -1.0, np.float32)
    dlB = np.full((NCORE, NCHB * P), -1.0, np.float32)
    # per-edge-slot source x and deg (layer-1 aggregates rank-2 u = x*dinv
    # directly from these, no gather): A slots then B slots
    xes = np.zeros((NCORE, NCH * P, 2), np.float32)
    deges = np.ones((NCORE, NCH * P), np.float32)
    so_src = src_g[order]
    isA = so_half == 0
    slotA = offA[so_blk[isA]] * P + pos_in_run[isA]
    idxA[so_core[isA], slotA] = so_row[isA].astype(np.int16)
    dlA[so_core[isA], slotA] = so_dl[isA]
    xes[so_core[isA], slotA] = x[so_src[isA]]
    deges[so_core[isA], slotA] = deg[so_src[isA]]
    isB = ~isA
    slotB = offB[so_blk[isB]] * P + pos_in_run[isB]
    idxB[so_core[isB], slotB] = (so_row[isB] - HALF).astype(np.int16)
    dlB[so_core[isB], slotB] = so_dl[isB]
    xes[so_core[isB], NCHA * P + slotB] = x[so_src[isB]]
    deges[so_core[isB], NCHA * P + slotB] = deg[so_src[isB]]

    # pooling masks
    gonehot = np.zeros((NCORE, NBLK * P, GPC), np.float32)
    gmask = np.zeros((NCORE, GPC, NBLK), np.float32)
    for g in range(n_graphs):
        k, gl = g // GPC, g % GPC
        b0 = loc_base[g] // P
        gmask[k, gl, b0:b0 + gblk[g]] = 1.0
        gonehot[k, loc_base[g]:loc_base[g] + gsz[g], gl] = 1.0

    cores = []
    for k in range(NCORE):
        cores.append(dict(
            idxA=wrap16(idxA[k]),                                 # [128, NCHA*8] i16
            idxB=wrap16(idxB[k]),
            dlA=np.ascontiguousarray(
                dlA[k].reshape(NCHA, P).T).astype(ml_dtypes.bfloat16),  # [128, NCHA]
            dlB=np.ascontiguousarray(
                dlB[k].reshape(NCHB, P).T).astype(ml_dtypes.bfloat16),
            deg_pm=deg_pm[k],                                     # [128, NBLK] f32
            degrow_rep=np.tile(degrow[k][None, :], (P, 1)).astype(ml_dtypes.bfloat16),
            deg_pm2_own=np.repeat(deg_pm[k], 2, axis=1).astype(ml_dtypes.bfloat16),  # [128, 2*NBLK]
            xnm2_own=xnm2[k].astype(ml_dtypes.bfloat16),          # [128, 2*NBLK]
            xes=np.ascontiguousarray(
                xes[k].reshape(NCH, P, 2).transpose(1, 0, 2)).astype(ml_dtypes.bfloat16),  # [128, NCH, 2]
            deges=np.ascontiguousarray(
                deges[k].reshape(NCH, P).T).astype(ml_dtypes.bfloat16),  # [128, NCH]
            gonehot=np.ascontiguousarray(
                gonehot[k].reshape(NBLK, P, GPC).transpose(1, 0, 2)).astype(np.float32),
            gmask=np.tile(gmask[k].reshape(1, GPC * NBLK), (P, 1)).astype(np.float32),
            gvalid=np.tile((gsz[k * GPC:(k + 1) * GPC] > 0).astype(np.float32), (P, 1)),
            cntrep=np.tile(gsz[k * GPC:(k + 1) * GPC].astype(np.float32), (P, 1)),
        ))

    meta = dict(NBLK=NBLK, NMAXP=NMAXP, HALF=HALF, NCHA=NCHA, NCHB=NCHB,
                cpA=cpA.astype(int), cpB=cpB.astype(int),
                offA=offA.astype(int), offB=offB.astype(int),
                gsz=gsz, cores=cores)
    return meta


def build(meta, GBLK=6, SINGLE_PACKET=False, SKIP_AG=False, AGROWS=None, BARRIER=False):
    NBLK, NMAXP, HALF = meta["NBLK"], meta["NMAXP"], meta["HALF"]
    NCHA, NCHB = meta["NCHA"], meta["NCHB"]
    cpA, cpB, offA, offB = meta["cpA"], meta["cpB"], meta["offA"], meta["offB"]
    NTAB = NCORE * NMAXP
    ngrp = (NBLK + GBLK - 1) // GBLK
    groups = []
    for g in range(ngrp):
        b0, b1 = g * GBLK, min((g + 1) * GBLK, NBLK)
        groups.append((b0, b1, int(offA[b0]), int(offA[b1]), int(offB[b0]), int(offB[b1])))
    GMAXA = max(a1 - a0 for (_, _, a0, a1, _, _) in groups)
    GMAXB = max(bb1 - bb0 for (_, _, _, _, bb0, bb1) in groups)

    nc = bacc.Bacc(None, target_bir_lowering=False, num_devices=NCORE if BARRIER else None)

    # ---- IO ----
    idxA_d = nc.dram_tensor("idxA", [128, NCHA * 8], I16, kind="ExternalInput")
    idxB_d = nc.dram_tensor("idxB", [128, NCHB * 8], I16, kind="ExternalInput")
    dlA_d = nc.dram_tensor("dlA", [128, NCHA], BF16, kind="ExternalInput")
    dlB_d = nc.dram_tensor("dlB", [128, NCHB], BF16, kind="ExternalInput")
    colidx_d = nc.dram_tensor("colidx", [128, 128], BF16, kind="ExternalInput")
    identb_d = nc.dram_tensor("identb", [128, 128], BF16, kind="ExternalInput")
    ident_d = nc.dram_tensor("ident", [128, 128], F32, kind="ExternalInput")
    W1_d = nc.dram_tensor("W1", [2, H], F32, kind="ExternalInput")
    W2_d = nc.dram_tensor("W2", [H, H], F32, kind="ExternalInput")
    W3_d = nc.dram_tensor("W3", [H, H], F32, kind="ExternalInput")
    Wo_d = nc.dram_tensor("Wo", [H, 2, OC], F32, kind="ExternalInput")
    bo_d = nc.dram_tensor("bo", [GPC, OC], F32, kind="ExternalInput")
    bvec_d = nc.dram_tensor("bvec", [128, 2], F32, kind="ExternalInput")
    brep3_d = nc.dram_tensor("brep3", [128, H], F32, kind="ExternalInput")
    goh_d = nc.dram_tensor("gonehot", [128, NBLK, GPC], F32, kind="ExternalInput")
    gmask_d = nc.dram_tensor("gmask", [128, GPC * NBLK], F32, kind="ExternalInput")
    gvalid_d = nc.dram_tensor("gvalid", [128, GPC], F32, kind="ExternalInput")
    cntrep_d = nc.dram_tensor("cntrep", [128, GPC], F32, kind="ExternalInput")
    deg_pm_d = nc.dram_tensor("deg_pm", [128, NBLK], F32, kind="ExternalInput")
    degrow_rep_d = nc.dram_tensor("degrow_rep", [128, NMAXP], BF16, kind="ExternalInput")
    deg_pm2_own_d = nc.dram_tensor("deg_pm2_own", [128, 2 * NBLK], BF16, kind="ExternalInput")
    xnm2_own_d = nc.dram_tensor("xnm2_own", [128, 2 * NBLK], BF16, kind="ExternalInput")
    NCH = NCHA + NCHB
    xes_d = nc.dram_tensor("xes", [128, NCH, 2], BF16, kind="ExternalInput")
    deges_d = nc.dram_tensor("deges", [128, NCH], BF16, kind="ExternalInput")
    out_d = nc.dram_tensor("out", [GPC, OC], F32, kind="ExternalOutput")

    NB2 = NBLK // 2
    R0 = NCORE * NB2 * P
    shardA_int = [nc.dram_tensor(f"shardA{L}", [NB2 * P, H], BF16) for L in (2, 3)]
    shardB_int = [nc.dram_tensor(f"shardB{L}", [(NBLK - NB2) * P, H], BF16) for L in (2, 3)]
    tableA_int = [nc.dram_tensor(f"tableA{L}", [R0, H], BF16, addr_space="Shared") for L in (2, 3)]
    tableB_int = [nc.dram_tensor(f"tableB{L}", [NTAB - R0, H], BF16, addr_space="Shared") for L in (2, 3)]

    with tile.TileContext(nc) as tc, ExitStack() as ctx:
        const = ctx.enter_context(tc.tile_pool(name="const", bufs=1))
        resid = ctx.enter_context(tc.tile_pool(name="resid", bufs=1))
        gap = ctx.enter_context(tc.tile_pool(name="gap", bufs=2))
        gbp = ctx.enter_context(tc.tile_pool(name="gbp", bufs=2))
        ohp = ctx.enter_context(tc.tile_pool(name="ohp", bufs=2))
        wk = ctx.enter_context(tc.tile_pool(name="wk", bufs=3))
        scr = ctx.enter_context(tc.tile_pool(name="scr", bufs=1))
        aggps = ctx.enter_context(tc.tile_pool(name="aggps", bufs=3, space="PSUM"))
        prepps = ctx.enter_context(tc.tile_pool(name="prepps", bufs=2, space="PSUM"))
        tps = ctx.enter_context(tc.tile_pool(name="tps", bufs=2, space="PSUM"))
        poolps = ctx.enter_context(tc.tile_pool(name="poolps", bufs=1, space="PSUM"))

        nc.gpsimd.load_library(library_config.mlp)
        if BARRIER:
            nc.all_core_barrier()

        def load_const(dram, shape, dt):
            t = const.tile(shape, dt, tag=dram.name)
            nc.sync.dma_start(t[:], dram[:])
            return t

        idxA_t = load_const(idxA_d, [128, NCHA * 8], I16)
        idxB_t = load_const(idxB_d, [128, NCHB * 8], I16)
        dlA_t = load_const(dlA_d, [128, NCHA], BF16)
        dlB_t = load_const(dlB_d, [128, NCHB], BF16)
        colidx_t = load_const(colidx_d, [128, 128], BF16)
        identb_t = load_const(identb_d, [128, 128], BF16)
        ident_t = load_const(ident_d, [128, 128], F32)
        W1_t = load_const(W1_d, [2, H], F32)
        W2_t = load_const(W2_d, [H, H], F32)
        W3_t = load_const(W3_d, [H, H], F32)
        Wo_t = load_const(Wo_d, [H, 2, OC], F32)
        bo_t = load_const(bo_d, [GPC, OC], F32)
        bvec_t = load_const(bvec_d, [128, 2], F32)
        brep3_t = load_const(brep3_d, [128, H], F32)
        goh_t = load_const(goh_d, [128, NBLK, GPC], F32)
        gmask_t = load_const(gmask_d, [128, GPC * NBLK], F32)
        gvalid_t = load_const(gvalid_d, [128, GPC], F32)
        cntrep_t = load_const(cntrep_d, [128, GPC], F32)
        deg_pm_t = load_const(deg_pm_d, [128, NBLK], F32)
        degrow_rep_t = load_const(degrow_rep_d, [128, NMAXP], BF16)
        deg_pm2_own_t = load_const(deg_pm2_own_d, [128, 2 * NBLK], BF16)
        xnm2_own_t = load_const(xnm2_own_d, [128, 2 * NBLK], BF16)
        xes_t = load_const(xes_d, [128, NCH, 2], BF16)
        deges_t = load_const(deges_d, [128, NCH], BF16)

        # ---- P1: rsqrt (reciprocal+sqrt) + u-table ----
        def rsqrt(out_tile, in_tile, shape, tmp_tag, dt=F32):
            rec = scr.tile(shape, dt, tag=tmp_tag)
            with nc.allow_low_precision(reason="deg is exact in bf16; dinv tol ~0.4%"):
                nc.vector.reciprocal(rec[:], in_tile[:])
            nc.scalar.activation(out_tile[:], rec[:], AF.Sqrt)

        dinv_pm = resid.tile([128, NBLK], F32, tag="dinv_pm")
        rsqrt(dinv_pm, deg_pm_t, [128, NBLK], "r1")
        dinvrep = resid.tile([128, NMAXP], BF16, tag="dinvrep")
        rsqrt(dinvrep, degrow_rep_t, [128, NMAXP], "r2", dt=BF16)

        d2o = scr.tile([128, 2 * NBLK], BF16, tag="d2o")
        rsqrt(d2o, deg_pm2_own_t, [128, 2 * NBLK], "r3", dt=BF16)
        u_own = resid.tile([128, 2 * NBLK], BF16, tag="u_own")
        nc.vector.tensor_tensor(u_own[:], xnm2_own_t[:], d2o[:], OP.mult)

        # layer-1 per-edge-slot u = x_src * dinv_src (no gather needed)
        dinv_es = scr.tile([128, NCH], BF16, tag="dinv_es")
        rsqrt(dinv_es, deges_t, [128, NCH], "r4", dt=BF16)
        ues = resid.tile([128, NCH, 2], BF16, tag="ues")
        nc.vector.tensor_tensor(
            ues[:], xes_t[:], dinv_es[:, :, None].broadcast_to((128, NCH, 2)), OP.mult)

        W1b = const.tile([2, H], BF16, tag="W1b")
        nc.vector.tensor_copy(W1b[:], W1_t[:])
        W2b = const.tile([H, H], BF16, tag="W2b")
        nc.vector.tensor_copy(W2b[:], W2_t[:])
        W3b = const.tile([H, H], BF16, tag="W3b")
        nc.vector.tensor_copy(W3b[:], W3_t[:])

        sbuild = resid.tile([128, NBLK, H], BF16, tag="sbuild")
        part = resid.tile([128, NBLK, H], BF16, tag="part")
        meanp = poolps.tile([128, GPC], F32, tag="meanp")
        pmax = resid.tile([128, NBLK], F32, tag="pmax")

        def build_oh(a0, a1, b0c, b1c):
            """One-hot tile for a group: A chunks then B chunks."""
            na, nb = a1 - a0, b1c - b0c
            oh = ohp.tile([128, GMAXA + GMAXB, 128], BF16, tag="oh")
            if na:
                cb = colidx_t[:, None, :].broadcast_to((128, na, 128))
                db = dlA_t[:, a0:a1, None].broadcast_to((128, na, 128))
                nc.vector.tensor_tensor(oh[:, :na, :], cb, db, OP.is_equal)
            if nb:
                cb = colidx_t[:, None, :].broadcast_to((128, nb, 128))
                db = dlB_t[:, b0c:b1c, None].broadcast_to((128, nb, 128))
                nc.vector.tensor_tensor(oh[:, na:na + nb, :], cb, db, OP.is_equal)
            return oh

        def gather_group(tabA, tabB, width, a0, a1, b0c, b1c, gmaxa, gmaxb):
            na, nb = a1 - a0, b1c - b0c
            gA = gB = None
            if na:
                gA = gap.tile([128, gmaxa, width], BF16, tag="gA")
                nA = na * 128
                nc.gpsimd.dma_gather(
                    gA[:, :na, :], tabA[:, :],
                    idxA_t[:, a0 * 8: a1 * 8], nA, nA, width,
                    single_packet=SINGLE_PACKET,
                )
            if nb:
                gB = gbp.tile([128, gmaxb, width], BF16, tag="gB")
                nB = nb * 128
                nc.gpsimd.dma_gather(
                    gB[:, :nb, :], tabB[:, :],
                    idxB_t[:, b0c * 8: b1c * 8], nB, nB, width,
                    single_packet=SINGLE_PACKET,
                )
            return gA, gB

        def half_ag(li, second=False):
            if not second:
                nc.sync.dma_start(
                    shardA_int[li].rearrange("(b p) h -> p b h", p=128)[:, :, :],
                    sbuild[:, 0:NB2, :],
                )
                nc.gpsimd.collective_compute(
                    "AllGather", OP.bypass, replica_groups=[list(range(NCORE))],
                    ins=[shardA_int[li][:]], outs=[tableA_int[li][:]],
                )
            else:
                nc.sync.dma_start(
                    shardB_int[li].rearrange("(b p) h -> p b h", p=128)[:, :, :],
                    sbuild[:, NB2:NBLK, :],
                )
                nc.gpsimd.collective_compute(
                    "AllGather", OP.bypass, replica_groups=[list(range(NCORE))],
                    ins=[shardB_int[li][:]], outs=[tableB_int[li][:]],
                )

        # ================= Layer 1 (transposed, rank-2) + prepare L2 ========
        for (b0, b1, a0, a1, bb0, bb1) in groups:
            oh = build_oh(a0, a1, bb0, bb1)
            for b in range(b0, b1):
                # aggUT [2, d] = u_own_blk^T + sum_chunks ues^T onehot-summed
                aggUT_full = aggps.tile([128, 128], F32, tag="agg")
                aggUT = aggUT_full[0:2, :]
                mms = [("self", None)]
                mms += [("A", c) for c in range(int(offA[b]) - a0, int(offA[b + 1]) - a0)]
                mms += [("B", c) for c in range(int(offB[b]) - bb0, int(offB[b + 1]) - bb0)]
                nA = a1 - a0
                for i, (kind, c) in enumerate(mms):
                    st, sp = (i == 0), (i == len(mms) - 1)
                    if kind == "self":
                        nc.tensor.matmul(aggUT, u_own[:, b * 2:(b + 1) * 2],
                                         identb_t[:], start=st, stop=sp)
                    elif kind == "A":
                        nc.tensor.matmul(aggUT, ues[:, a0 + c, :], oh[:, c, :],
                                         start=st, stop=sp)
                    else:
                        nc.tensor.matmul(aggUT, ues[:, NCHA + bb0 + c, :],
                                         oh[:, nA + c, :], start=st, stop=sp)
                cU = wk.tile([2, 128], BF16, tag="cU")
                nc.scalar.copy(cU[:], aggUT)
                hpreT = tps.tile([H, 128], F32, tag="tp")
                nc.tensor.matmul(hpreT[:], W1b[:], cU[:], start=True, stop=True)
                e1 = wk.tile([128, 128], BF16, tag="e1")
                nc.vector.tensor_tensor(
                    e1[:], hpreT[:], dinvrep[:, b * 128:(b + 1) * 128], OP.mult)
                hT = wk.tile([128, 128], BF16, tag="hT")
                nc.scalar.activation(hT[:], e1[:], AF.Tanh, bias=bvec_t[:, 0:1])
                # prepare L2: t1 = (h @ W2) * dinv -> sbuild
                pp = prepps.tile([128, H], F32, tag="pp")
                nc.tensor.matmul(pp[:], hT[:], W2b[:], start=True, stop=True)
                nc.vector.tensor_scalar(
                    sbuild[:, b, :], pp[:], dinv_pm[:, b:b + 1], None, OP.mult)
                if b == NB2 - 1 and SKIP_AG not in (True, "first"):
                    half_ag(0)

        if SKIP_AG in (True, "first"):
            nc.sync.dma_start(
                shardA_int[0].rearrange("(b p) h -> p b h", p=128)[:, :, :],
                sbuild[:, 0:NB2, :],
            )
        else:
            half_ag(0, second=True)

        # ================= Layer 2 (transposed) + prepare L3 ================
        # pass 1: self + A-half chunks -> SBUF partial (overlaps the B half-AG)
        for (b0, b1, a0, a1, bb0, bb1) in groups:
            gA, _ = gather_group(tableA_int[0], None, H, a0, a1, bb0, bb0, GMAXA, GMAXB)
            oh = build_oh(a0, a1, bb0, bb0)
            for b in range(b0, b1):
                aggT = aggps.tile([128, 128], F32, tag="agg")
                mms = [("self", None)]
                mms += [("A", c) for c in range(int(offA[b]) - a0, int(offA[b + 1]) - a0)]
                for i, (kind, c) in enumerate(mms):
                    st, sp = (i == 0), (i == len(mms) - 1)
                    if kind == "self":
                        nc.tensor.matmul(aggT[:], sbuild[:, b, :], identb_t[:],
                                         start=st, stop=sp)
                    else:
                        nc.tensor.matmul(aggT[:], gA[:, c, :], oh[:, c, :],
                                         start=st, stop=sp)
                nc.scalar.copy(part[:, b, :], aggT[:])
        # pass 2: B-half chunks + partial -> epilogue + prepare L3
        for (b0, b1, a0, a1, bb0, bb1) in groups:
            _, gB = gather_group(None, tableB_int[0], H, a0, a0, bb0, bb1, GMAXA, GMAXB)
            oh = build_oh(a0, a0, bb0, bb1)
            for b in range(b0, b1):
                nB = int(offB[b + 1]) - int(offB[b])
                if nB:
                    aggT = aggps.tile([128, 128], F32, tag="agg")
                    for i, c in enumerate(range(int(offB[b]) - bb0, int(offB[b + 1]) - bb0)):
                        nc.tensor.matmul(aggT[:], gB[:, c, :], oh[:, c, :],
                                         start=(i == 0), stop=(i == nB - 1))
                    s1 = wk.tile([128, 128], F32, tag="s1")
                    nc.vector.tensor_tensor(s1[:], aggT[:], part[:, b, :], OP.add)
                    src_agg = s1
                else:
                    src_agg = part[:, b, :]
                e1 = wk.tile([128, 128], BF16, tag="e1")
                nc.vector.tensor_tensor(
                    e1[:], src_agg if nB == 0 else s1[:],
                    dinvrep[:, b * 128:(b + 1) * 128], OP.mult)
                hT = wk.tile([128, 128], BF16, tag="hT")
                nc.scalar.activation(hT[:], e1[:], AF.Tanh, bias=bvec_t[:, 1:2])
                pp = prepps.tile([128, H], F32, tag="pp")
                nc.tensor.matmul(pp[:], hT[:], W3b[:], start=True, stop=True)
                nc.vector.tensor_scalar(
                    sbuild[:, b, :], pp[:], dinv_pm[:, b:b + 1], None, OP.mult)
                if b == NB2 - 1 and SKIP_AG not in (True, "second"):
                    half_ag(1)

        if SKIP_AG in (True, "second"):
            nc.sync.dma_start(
                shardA_int[1].rearrange("(b p) h -> p b h", p=128)[:, :, :],
                sbuild[:, 0:NB2, :],
            )
        else:
            half_ag(1, second=True)

        # ================= Layer 3 (node-major) + pooling ===================
        for (b0, b1, a0, a1, bb0, bb1) in groups:
            gA, gB = gather_group(tableA_int[1], tableB_int[1], H, a0, a1, bb0, bb1, GMAXA, GMAXB)
            oh = build_oh(a0, a1, bb0, bb1)
            for b in range(b0, b1):
                agg = aggps.tile([128, H], F32, tag="agg")
                mms = [("self", None)]
                mms += [("A", c) for c in range(int(offA[b]) - a0, int(offA[b + 1]) - a0)]
                mms += [("B", c) for c in range(int(offB[b]) - bb0, int(offB[b + 1]) - bb0)]
                nA = a1 - a0
                for i, (kind, c) in enumerate(mms):
                    st, sp = (i == 0), (i == len(mms) - 1)
                    if kind == "self":
                        nc.tensor.matmul(agg[:], identb_t[:], sbuild[:, b, :],
                                         start=st, stop=sp)
                    elif kind == "A":
                        nc.tensor.matmul(agg[:], oh[:, c, :], gA[:, c, :],
                                         start=st, stop=sp)
                    else:
                        nc.tensor.matmul(agg[:], oh[:, nA + c, :], gB[:, c, :],
                                         start=st, stop=sp)
                e2 = wk.tile([128, H], F32, tag="e2")
                nc.vector.scalar_tensor_tensor(
                    e2[:], agg[:], dinv_pm[:, b:b + 1], brep3_t[:], OP.mult, OP.add)
                hblk = wk.tile([128, H], F32, tag="hblk")
                nc.scalar.activation(hblk[:], e2[:], AF.Tanh)
                # mean pool accumulate; max pool via PE transpose + free reduce
                nc.tensor.matmul(meanp[:], hblk[:], goh_t[:, b, :],
                                 start=(b == 0), stop=(b == NBLK - 1))
                tp = tps.tile([128, H], F32, tag="tp")
                nc.tensor.transpose(tp[:], hblk[:], ident_t[:])
                nc.vector.tensor_reduce(
                    pmax[:, b:b + 1], tp[:], mybir.AxisListType.X, OP.max)

        # ---- pooling tail + head ----
        p2 = resid.tile([128, NBLK], F32, tag="p2")
        nc.vector.tensor_scalar(p2[:], pmax[:], 2.0, None, OP.add)
        mg = wk.tile([128, GPC, NBLK], F32, tag="mg")
        nc.vector.tensor_tensor(
            mg[:], p2[:, None, :].broadcast_to((128, GPC, NBLK)),
            gmask_t[:].rearrange("p (g b) -> p g b", g=GPC), OP.mult)
        mxT = resid.tile([128, GPC], F32, tag="mxT")
        nc.vector.tensor_reduce(
            mxT[:, :, None], mg[:], mybir.AxisListType.X, OP.max)
        mxT2 = resid.tile([128, GPC], F32, tag="mxT2")
        nc.vector.scalar_tensor_tensor(
            mxT2[:], mxT[:], -2.0, gvalid_t[:], OP.add, OP.mult)

        cmax = wk.tile([128, GPC], F32, tag="cmax")
        nc.vector.tensor_scalar(cmax[:], cntrep_t[:], 1.0, None, OP.max)
        crec = wk.tile([128, GPC], F32, tag="crec")
        nc.vector.reciprocal(crec[:], cmax[:])
        meanT = wk.tile([128, GPC], F32, tag="meanT")
        nc.vector.tensor_tensor(meanT[:], meanp[:], crec[:], OP.mult)

        headp_full = prepps.tile([128, H], F32, tag="pp")
        headp = headp_full[0:GPC, 0:OC]
        nc.tensor.matmul(headp, mxT2[:], Wo_t[:, 0, :], start=True, stop=False)
        nc.tensor.matmul(headp, meanT[:], Wo_t[:, 1, :], start=False, stop=True)
        hsum = wk.tile([GPC, OC], F32, tag="hsum")
        nc.vector.tensor_tensor(hsum[:], headp, bo_t[:], OP.add)
        ofin = wk.tile([GPC, OC], F32, tag="ofin")
        nc.scalar.activation(ofin[:], hsum[:], AF.Tanh)
        nc.sync.dma_start(out_d[:], ofin[:])

    nc.compile()
    return nc


def make_in_maps(meta, inputs):
    colidx = np.tile(np.arange(128, dtype=np.float32), (128, 1)).astype(ml_dtypes.bfloat16)
    identb = np.eye(128, dtype=np.float32).astype(ml_dtypes.bfloat16)
    bvec = np.stack([np.asarray(inputs["b1"], np.float32),
                     np.asarray(inputs["b2"], np.float32)], axis=1)  # [128, 2]
    brep3 = np.tile(np.asarray(inputs["b3"], np.float32), (P, 1))
    bo_t = np.tile(np.asarray(inputs["bo"], np.float32), (GPC, 1))
    Wo = np.asarray(inputs["Wo"], np.float32)
    maps = []
    for c in meta["cores"]:
        maps.append({
            "idxA": c["idxA"], "idxB": c["idxB"],
            "dlA": c["dlA"], "dlB": c["dlB"],
            "colidx": colidx, "identb": identb, "ident": np.eye(128, dtype=np.float32),
            "W1": np.asarray(inputs["W1"], np.float32),
            "W2": np.asarray(inputs["W2"], np.float32),
            "W3": np.asarray(inputs["W3"], np.float32),
            "Wo": np.ascontiguousarray(np.stack([Wo[:H], Wo[H:]], axis=1)),
            "bo": bo_t, "bvec": bvec, "brep3": brep3,
            "gonehot": c["gonehot"], "gmask": c["gmask"], "gvalid": c["gvalid"],
            "cntrep": c["cntrep"],
            "deg_pm": c["deg_pm"], "degrow_rep": c["degrow_rep"],
            "deg_pm2_own": c["deg_pm2_own"], "xnm2_own": c["xnm2_own"],
            "xes": c["xes"], "deges": c["deges"],
        })
    return maps


_CACHE = {}


def kernel(x, edge_index, batch, W1, b1, W2, b2, W3, b3, Wo, bo):
    x = np.asarray(x, np.float32)
    edge_index = np.asarray(edge_index)
    batch = np.asarray(batch)
    meta = prep(x, edge_index, batch, 64)
    key = (meta["NBLK"], meta["NCHA"], meta["NCHB"])
    if key not in _CACHE:
        _CACHE[key] = build(meta)
    nc = _CACHE[key]
    inputs = dict(W1=W1, b1=b1, W2=W2, b2=b2, W3=W3, b3=b3, Wo=Wo, bo=bo)
    in_maps = make_in_maps(meta, inputs)
    res = run_bass_kernel_spmd(nc, in_maps, core_ids=list(range(8)), trace=False)
    out = np.concatenate([res.results[k]["out"] for k in range(8)], 0)
    return np.ascontiguousarray(out, dtype=np.float32)


# revision 46
# speedup vs baseline: 1.0284x; 1.0284x over previous
"""Self-contained Trainium2 Bass kernel for nn_GCNMagnetModel (3-layer GCN,
N=50000 nodes, E=600000 edges, H=128, 64 graphs, 8 NeuronCores, SPMD 1 NEFF).

Sharding: nodes/edges sharded across 8 cores by graph id (graphs 8k..8k+7 ->
core k; graph-block-aligned node layout so both pools are core-local).

Dataflow (v2):
- Host computes all index layout AND integer degree counts (bincount); the
  device does rsqrt and every other FP op on tensor values.
- Layer 1 is rank-2: agg((x@W1)*dinv) == agg(x*dinv) @ W1, so layer-1 message
  passing gathers 2-wide u-rows (4B) instead of 128-wide table rows; the
  layer-1 table build and its AllGather disappear.
- Layers 2/3 gather bf16 table rows of t1 = (h@W)*dinv_src from an
  AllGather'd table; per dst-block-of-128 segment-sum via one-hot matmuls.
- The GCN self-loop term is one identity matmul from the SBUF-resident
  sbuild (t1) tile per block - no separate bstar machinery.
- Layers 1/2 aggregate in TRANSPOSED orientation (aggT[H,d] with the gathered
  chunk as the stationary operand) so h feeds the next layer's prepare matmul
  with no PE transpose; prepare is fused into the same block iteration, so no
  resident hT buffer exists. Layer 3 aggregates node-major for pooling.
- Per-(block,half) chunk counts are the max over the 8 cores (SPMD shapes)
  instead of a global max, roughly halving gather volume and one-hot builds.
- Each table AllGather is split into two contiguous half-shard collectives on
  SEPARATE DRAM tensors (half-major row numbering: region0 = all cores'
  blocks < NBLK/2, region1 = rest; the region boundary doubles as the int16
  A/B gather split). The first half-AG issues mid-loop so its transfer
  overlaps the remaining blocks' compute, and the next layer's A-half
  gathers depend only on it - the second half-AG overlaps their gathers.

kernel(**inputs) -> [64, 41] float32.
"""
import numpy as np
import ml_dtypes
from contextlib import ExitStack

import concourse.tile as tile
import concourse.mybir as mybir
from concourse import bacc
from concourse import library_config
from concourse.bass_utils import run_bass_kernel_spmd

NCORE = 8
P = 128
GPC = 8  # graphs per core
H = 128
OC = 41

F32 = mybir.dt.float32
BF16 = mybir.dt.bfloat16
I16 = mybir.dt.int16
AF = mybir.ActivationFunctionType
OP = mybir.AluOpType


def wrap16(v):  # [n] -> [128, n/16]: idx[i%16, i//16] tiled 8x
    a = v.reshape(-1, 16).T
    return np.tile(a, (8, 1)).copy()


def prep(x, edge_index, batch, n_graphs=64):
    N = x.shape[0]
    x = np.asarray(x, np.float32)
    batch = np.asarray(batch)
    src_g, dst_g = np.asarray(edge_index[0]), np.asarray(edge_index[1])
    E = src_g.shape[0]

    gstart = np.searchsorted(batch, np.arange(n_graphs), side="left")
    gend = np.searchsorted(batch, np.arange(n_graphs), side="right")
    gsz = gend - gstart

    gblk = np.maximum((gsz + P - 1) // P, 1)
    nblk_core = [int(gblk[k * GPC:(k + 1) * GPC].sum()) for k in range(NCORE)]
    NBLK = max(nblk_core)
    NMAXP = NBLK * P

    loc_base = np.zeros(n_graphs, np.int64)
    for g in range(n_graphs):
        if g % GPC == 0:
            loc_base[g] = 0
        else:
            loc_base[g] = loc_base[g - 1] + gblk[g - 1] * P
    node_core = batch // GPC
    node_loc = loc_base[batch] + (np.arange(N) - gstart[batch])
    # half-major rows: region0 = all cores' blocks < NB2 (k-major), region1 =
    # the rest; both regions contiguous so each half-shard AllGather is a
    # contiguous ins/outs pair, and region0 boundary doubles as the int16
    # A/B gather-table split.
    NB2 = NBLK // 2
    R0 = NCORE * NB2 * P
    node_b = node_loc // P
    node_p = node_loc % P
    node_row = np.where(
        node_b < NB2,
        node_core * (NB2 * P) + node_b * P + node_p,
        R0 + node_core * ((NBLK - NB2) * P) + (node_b - NB2) * P + node_p)

    HALF = R0
    assert HALF < 32768 and NCORE * NMAXP - HALF < 32768, f"HALF={HALF}"

    # host degree counts (integer index work); +1 self loop
    deg = np.bincount(dst_g, minlength=N).astype(np.float32) + 1.0

    # per-core padded layouts
    # deg_pm[k][p, b] ; degrow[k][loc] ; x node-major interleaved
    deg_pm = np.ones((NCORE, P, NBLK), np.float32)
    degrow = np.ones((NCORE, NMAXP), np.float32)
    xnm2 = np.zeros((NCORE, P, NBLK * 2), np.float32)
    pidx = (node_loc % P).astype(np.int64)
    bidx = (node_loc // P).astype(np.int64)
    deg_pm[node_core, pidx, bidx] = deg
    degrow[node_core, node_loc] = deg
    xnm2[node_core, pidx, bidx * 2] = x[:, 0]
    xnm2[node_core, pidx, bidx * 2 + 1] = x[:, 1]

    # edges -> (core, blk, half); chunk counts = per-(blk,half) max over cores
    e_core = node_core[dst_g]
    e_dstloc = node_loc[dst_g]
    e_blk = e_dstloc // P
    e_dl = (e_dstloc % P).astype(np.float32)
    e_row = node_row[src_g]
    e_half = (e_row >= HALF).astype(np.int64)

    cnts = np.zeros((NCORE, NBLK, 2), np.int64)
    np.add.at(cnts, (e_core, e_blk, e_half), 1)
    cp = (cnts.max(axis=0) + P - 1) // P          # [NBLK, 2] chunks
    cpA, cpB = cp[:, 0], cp[:, 1]
    offA = np.r_[0, np.cumsum(cpA)]               # [NBLK+1]
    offB = np.r_[0, np.cumsum(cpB)]
    NCHA, NCHB = int(offA[-1]), int(offB[-1])

    # slot assignment: sort edges by (core, half, blk), fill runs
    order = np.lexsort((e_blk, e_half, e_core))
    so_core, so_blk, so_half = e_core[order], e_blk[order], e_half[order]
    so_row, so_dl = e_row[order], e_dl[order]
    key = (so_core * 2 + so_half) * NBLK + so_blk
    runstart = np.r_[0, np.flatnonzero(np.diff(key)) + 1]
    runid = np.zeros(E, np.int64)
    runid[runstart[1:]] = 1
    runid = np.cumsum(runid)
    pos_in_run = np.arange(E) - runstart[runid]

    NCH = NCHA + NCHB
    idxA = np.zeros((NCORE, NCHA * P), np.int16)
    idxB = np.zeros((NCORE, NCHB * P), np.int16)
    dlA = np.full((NCORE, NCHA * P), -1.0, np.float32)
    dlB = np.full((NCORE, NCHB * P), -1.0, np.float32)
    # per-edge-slot source x and deg (layer-1 aggregates rank-2 u = x*dinv
    # directly from these, no gather): A slots then B slots
    xes = np.zeros((NCORE, NCH * P, 2), np.float32)
    deges = np.ones((NCORE, NCH * P), np.float32)
    so_src = src_g[order]
    isA = so_half == 0
    slotA = offA[so_blk[isA]] * P + pos_in_run[isA]
    idxA[so_core[isA], slotA] = so_row[isA].astype(np.int16)
    dlA[so_core[isA], slotA] = so_dl[isA]
    xes[so_core[isA], slotA] = x[so_src[isA]]
    deges[so_core[isA], slotA] = deg[so_src[isA]]
    isB = ~isA
    slotB = offB[so_blk[isB]] * P + pos_in_run[isB]
    idxB[so_core[isB], slotB] = (so_row[isB] - HALF).astype(np.int16)
    dlB[so_core[isB], slotB] = so_dl[isB]
    xes[so_core[isB], NCHA * P + slotB] = x[so_src[isB]]
    deges[so_core[isB], NCHA * P + slotB] = deg[so_src[isB]]

    # pooling masks
    gonehot = np.zeros((NCORE, NBLK * P, GPC), np.float32)
    gmask = np.zeros((NCORE, GPC, NBLK), np.float32)
    for g in range(n_graphs):
        k, gl = g // GPC, g % GPC
        b0 = loc_base[g] // P
        gmask[k, gl, b0:b0 + gblk[g]] = 1.0
        gonehot[k, loc_base[g]:loc_base[g] + gsz[g], gl] = 1.0

    cores = []
    for k in range(NCORE):
        cores.append(dict(
            idxA=wrap16(idxA[k]),                                 # [128, NCHA*8] i16
            idxB=wrap16(idxB[k]),
            dlA=np.ascontiguousarray(
                dlA[k].reshape(NCHA, P).T).astype(ml_dtypes.bfloat16),  # [128, NCHA]
            dlB=np.ascontiguousarray(
                dlB[k].reshape(NCHB, P).T).astype(ml_dtypes.bfloat16),
            deg_pm=deg_pm[k],                                     # [128, NBLK] f32
            degrow_rep=np.tile(degrow[k][None, :], (P, 1)).astype(ml_dtypes.bfloat16),
            deg_pm2_own=np.repeat(deg_pm[k], 2, axis=1).astype(ml_dtypes.bfloat16),  # [128, 2*NBLK]
            xnm2_own=xnm2[k].astype(ml_dtypes.bfloat16),          # [128, 2*NBLK]
            xes=np.ascontiguousarray(
                xes[k].reshape(NCH, P, 2).transpose(1, 0, 2)).astype(ml_dtypes.bfloat16),  # [128, NCH, 2]
            deges=np.ascontiguousarray(
                deges[k].reshape(NCH, P).T).astype(ml_dtypes.bfloat16),  # [128, NCH]
            gonehot=np.ascontiguousarray(
                gonehot[k].reshape(NBLK, P, GPC).transpose(1, 0, 2)).astype(np.float32),
            gmask=np.tile(gmask[k].reshape(1, GPC * NBLK), (P, 1)).astype(np.float32),
            gvalid=np.tile((gsz[k * GPC:(k + 1) * GPC] > 0).astype(np.float32), (P, 1)),
            cntrep=np.tile(gsz[k * GPC:(k + 1) * GPC].astype(np.float32), (P, 1)),
        ))

    meta = dict(NBLK=NBLK, NMAXP=NMAXP, HALF=HALF, NCHA=NCHA, NCHB=NCHB,
                cpA=cpA.astype(int), cpB=cpB.astype(int),
                offA=offA.astype(int), offB=offB.astype(int),
                gsz=gsz, cores=cores)
    return meta


def build(meta, GBLK=6, SINGLE_PACKET=False, SKIP_AG=False, AGROWS=None, BARRIER=False):
    NBLK, NMAXP, HALF = meta["NBLK"], meta["NMAXP"], meta["HALF"]
    NCHA, NCHB = meta["NCHA"], meta["NCHB"]
    cpA, cpB, offA, offB = meta["cpA"], meta["cpB"], meta["offA"], meta["offB"]
    NTAB = NCORE * NMAXP
    ngrp = (NBLK + GBLK - 1) // GBLK
    groups = []
    for g in range(ngrp):
        b0, b1 = g * GBLK, min((g + 1) * GBLK, NBLK)
        groups.append((b0, b1, int(offA[b0]), int(offA[b1]), int(offB[b0]), int(offB[b1])))
    GMAXA = max(a1 - a0 for (_, _, a0, a1, _, _) in groups)
    GMAXB = max(bb1 - bb0 for (_, _, _, _, bb0, bb1) in groups)

    nc = bacc.Bacc(None, target_bir_lowering=False, num_devices=NCORE if BARRIER else None)

    # ---- IO ----
    idxA_d = nc.dram_tensor("idxA", [128, NCHA * 8], I16, kind="ExternalInput")
    idxB_d = nc.dram_tensor("idxB", [128, NCHB * 8], I16, kind="ExternalInput")
    dlA_d = nc.dram_tensor("dlA", [128, NCHA], BF16, kind="ExternalInput")
    dlB_d = nc.dram_tensor("dlB", [128, NCHB], BF16, kind="ExternalInput")
    colidx_d = nc.dram_tensor("colidx", [128, 128], BF16, kind="ExternalInput")
    identb_d = nc.dram_tensor("identb", [128, 128], BF16, kind="ExternalInput")
    ident_d = nc.dram_tensor("ident", [128, 128], F32, kind="ExternalInput")
    W1_d = nc.dram_tensor("W1", [2, H], F32, kind="ExternalInput")
    W2_d = nc.dram_tensor("W2", [H, H], F32, kind="ExternalInput")
    W3_d = nc.dram_tensor("W3", [H, H], F32, kind="ExternalInput")
    Wo_d = nc.dram_tensor("Wo", [H, 2, OC], F32, kind="ExternalInput")
    bo_d = nc.dram_tensor("bo", [GPC, OC], F32, kind="ExternalInput")
    bvec_d = nc.dram_tensor("bvec", [128, 2], F32, kind="ExternalInput")
    brep3_d = nc.dram_tensor("brep3", [128, H], F32, kind="ExternalInput")
    goh_d = nc.dram_tensor("gonehot", [128, NBLK, GPC], F32, kind="ExternalInput")
    gmask_d = nc.dram_tensor("gmask", [128, GPC * NBLK], F32, kind="ExternalInput")
    gvalid_d = nc.dram_tensor("gvalid", [128, GPC], F32, kind="ExternalInput")
    cntrep_d = nc.dram_tensor("cntrep", [128, GPC], F32, kind="ExternalInput")
    deg_pm_d = nc.dram_tensor("deg_pm", [128, NBLK], F32, kind="ExternalInput")
    degrow_rep_d = nc.dram_tensor("degrow_rep", [128, NMAXP], BF16, kind="ExternalInput")
    deg_pm2_own_d = nc.dram_tensor("deg_pm2_own", [128, 2 * NBLK], BF16, kind="ExternalInput")
    xnm2_own_d = nc.dram_tensor("xnm2_own", [128, 2 * NBLK], BF16, kind="ExternalInput")
    NCH = NCHA + NCHB
    xes_d = nc.dram_tensor("xes", [128, NCH, 2], BF16, kind="ExternalInput")
    deges_d = nc.dram_tensor("deges", [128, NCH], BF16, kind="ExternalInput")
    out_d = nc.dram_tensor("out", [GPC, OC], F32, kind="ExternalOutput")

    NB2 = NBLK // 2
    R0 = NCORE * NB2 * P
    shardA_int = [nc.dram_tensor(f"shardA{L}", [NB2 * P, H], BF16) for L in (2, 3)]
    shardB_int = [nc.dram_tensor(f"shardB{L}", [(NBLK - NB2) * P, H], BF16) for L in (2, 3)]
    tableA_int = [nc.dram_tensor(f"tableA{L}", [R0, H], BF16, addr_space="Shared") for L in (2, 3)]
    tableB_int = [nc.dram_tensor(f"tableB{L}", [NTAB - R0, H], BF16, addr_space="Shared") for L in (2, 3)]

    with tile.TileContext(nc) as tc, ExitStack() as ctx:
        const = ctx.enter_context(tc.tile_pool(name="const", bufs=1))
        resid = ctx.enter_context(tc.tile_pool(name="resid", bufs=1))
        gap = ctx.enter_context(tc.tile_pool(name="gap", bufs=2))
        gbp = ctx.enter_context(tc.tile_pool(name="gbp", bufs=2))
        ohp = ctx.enter_context(tc.tile_pool(name="ohp", bufs=2))
        wk = ctx.enter_context(tc.tile_pool(name="wk", bufs=3))
        scr = ctx.enter_context(tc.tile_pool(name="scr", bufs=1))
        aggps = ctx.enter_context(tc.tile_pool(name="aggps", bufs=3, space="PSUM"))
        prepps = ctx.enter_context(tc.tile_pool(name="prepps", bufs=2, space="PSUM"))
        tps = ctx.enter_context(tc.tile_pool(name="tps", bufs=2, space="PSUM"))
        poolps = ctx.enter_context(tc.tile_pool(name="poolps", bufs=1, space="PSUM"))

        nc.gpsimd.load_library(library_config.mlp)
        if BARRIER:
            nc.all_core_barrier()

        def load_const(dram, shape, dt):
            t = const.tile(shape, dt, tag=dram.name)
            nc.sync.dma_start(t[:], dram[:])
            return t

        idxA_t = load_const(idxA_d, [128, NCHA * 8], I16)
        idxB_t = load_const(idxB_d, [128, NCHB * 8], I16)
        dlA_t = load_const(dlA_d, [128, NCHA], BF16)
        dlB_t = load_const(dlB_d, [128, NCHB], BF16)
        colidx_t = load_const(colidx_d, [128, 128], BF16)
        identb_t = load_const(identb_d, [128, 128], BF16)
        ident_t = load_const(ident_d, [128, 128], F32)
        W1_t = load_const(W1_d, [2, H], F32)
        W2_t = load_const(W2_d, [H, H], F32)
        W3_t = load_const(W3_d, [H, H], F32)
        Wo_t = load_const(Wo_d, [H, 2, OC], F32)
        bo_t = load_const(bo_d, [GPC, OC], F32)
        bvec_t = load_const(bvec_d, [128, 2], F32)
        brep3_t = load_const(brep3_d, [128, H], F32)
        goh_t = load_const(goh_d, [128, NBLK, GPC], F32)
        gmask_t = load_const(gmask_d, [128, GPC * NBLK], F32)
        gvalid_t = load_const(gvalid_d, [128, GPC], F32)
        cntrep_t = load_const(cntrep_d, [128, GPC], F32)
        deg_pm_t = load_const(deg_pm_d, [128, NBLK], F32)
        degrow_rep_t = load_const(degrow_rep_d, [128, NMAXP], BF16)
        deg_pm2_own_t = load_const(deg_pm2_own_d, [128, 2 * NBLK], BF16)
        xnm2_own_t = load_const(xnm2_own_d, [128, 2 * NBLK], BF16)
        xes_t = load_const(xes_d, [128, NCH, 2], BF16)
        deges_t = load_const(deges_d, [128, NCH], BF16)

        # ---- P1: rsqrt (reciprocal+sqrt) + u-table ----
        def rsqrt(out_tile, in_tile, shape, tmp_tag, dt=F32):
            rec = scr.tile(shape, dt, tag=tmp_tag)
            with nc.allow_low_precision(reason="deg is exact in bf16; dinv tol ~0.4%"):
                nc.vector.reciprocal(rec[:], in_tile[:])
            nc.scalar.activation(out_tile[:], rec[:], AF.Sqrt)

        dinv_pm = resid.tile([128, NBLK], F32, tag="dinv_pm")
        rsqrt(dinv_pm, deg_pm_t, [128, NBLK], "r1")
        dinvrep = resid.tile([128, NMAXP], BF16, tag="dinvrep")
        rsqrt(dinvrep, degrow_rep_t, [128, NMAXP], "r2", dt=BF16)

        d2o = scr.tile([128, 2 * NBLK], BF16, tag="d2o")
        rsqrt(d2o, deg_pm2_own_t, [128, 2 * NBLK], "r3", dt=BF16)
        u_own = resid.tile([128, 2 * NBLK], BF16, tag="u_own")
        nc.vector.tensor_tensor(u_own[:], xnm2_own_t[:], d2o[:], OP.mult)

        # layer-1 per-edge-slot u = x_src * dinv_src (no gather needed)
        dinv_es = scr.tile([128, NCH], BF16, tag="dinv_es")
        rsqrt(dinv_es, deges_t, [128, NCH], "r4", dt=BF16)
        ues = resid.tile([128, NCH, 2], BF16, tag="ues")
        nc.vector.tensor_tensor(
            ues[:], xes_t[:], dinv_es[:, :, None].broadcast_to((128, NCH, 2)), OP.mult)

        W1b = const.tile([2, H], BF16, tag="W1b")
        nc.vector.tensor_copy(W1b[:], W1_t[:])
        W2b = const.tile([H, H], BF16, tag="W2b")
        nc.vector.tensor_copy(W2b[:], W2_t[:])
        W3b = const.tile([H, H], BF16, tag="W3b")
        nc.vector.tensor_copy(W3b[:], W3_t[:])

        sbuild = resid.tile([128, NBLK, H], BF16, tag="sbuild")
        part = resid.tile([128, NBLK, H], BF16, tag="part")
        meanp = poolps.tile([128, GPC], F32, tag="meanp")
        pmax = resid.tile([128, NBLK], F32, tag="pmax")

        def build_oh(a0, a1, b0c, b1c):
            """One-hot tile for a group: A chunks then B chunks."""
            na, nb = a1 - a0, b1c - b0c
            oh = ohp.tile([128, GMAXA + GMAXB, 128], BF16, tag="oh")
            if na:
                cb = colidx_t[:, None, :].broadcast_to((128, na, 128))
                db = dlA_t[:, a0:a1, None].broadcast_to((128, na, 128))
                nc.vector.tensor_tensor(oh[:, :na, :], cb, db, OP.is_equal)
            if nb:
                cb = colidx_t[:, None, :].broadcast_to((128, nb, 128))
                db = dlB_t[:, b0c:b1c, None].broadcast_to((128, nb, 128))
                nc.vector.tensor_tensor(oh[:, na:na + nb, :], cb, db, OP.is_equal)
            return oh

        def gather_group(tabA, tabB, width, a0, a1, b0c, b1c, gmaxa, gmaxb):
            na, nb = a1 - a0, b1c - b0c
            gA = gB = None
            if na:
                gA = gap.tile([128, gmaxa, width], BF16, tag="gA")
                nA = na * 128
                nc.gpsimd.dma_gather(
                    gA[:, :na, :], tabA[:, :],
                    idxA_t[:, a0 * 8: a1 * 8], nA, nA, width,
                    single_packet=SINGLE_PACKET,
                )
            if nb:
                gB = gbp.tile([128, gmaxb, width], BF16, tag="gB")
                nB = nb * 128
                nc.gpsimd.dma_gather(
                    gB[:, :nb, :], tabB[:, :],
                    idxB_t[:, b0c * 8: b1c * 8], nB, nB, width,
                    single_packet=SINGLE_PACKET,
                )
            return gA, gB

        def half_ag(li, second=False):
            if not second:
                nc.sync.dma_start(
                    shardA_int[li].rearrange("(b p) h -> p b h", p=128)[:, :, :],
                    sbuild[:, 0:NB2, :],
                )
                nc.gpsimd.collective_compute(
                    "AllGather", OP.bypass, replica_groups=[list(range(NCORE))],
                    ins=[shardA_int[li][:]], outs=[tableA_int[li][:]],
                )
            else:
                nc.sync.dma_start(
                    shardB_int[li].rearrange("(b p) h -> p b h", p=128)[:, :, :],
                    sbuild[:, NB2:NBLK, :],
                )
                nc.gpsimd.collective_compute(
                    "AllGather", OP.bypass, replica_groups=[list(range(NCORE))],
                    ins=[shardB_int[li][:]], outs=[tableB_int[li][:]],
                )

        # ================= Layer 1 (transposed, rank-2) + prepare L2 ========
        for (b0, b1, a0, a1, bb0, bb1) in groups:
            oh = build_oh(a0, a1, bb0, bb1)
            for b in range(b0, b1):
                # aggUT [2, d] = u_own_blk^T + sum_chunks ues^T onehot-summed
                aggUT_full = aggps.tile([128, 128], F32, tag="agg")
                aggUT = aggUT_full[0:2, :]
                mms = [("self", None)]
                mms += [("A", c) for c in range(int(offA[b]) - a0, int(offA[b + 1]) - a0)]
                mms += [("B", c) for c in range(int(offB[b]) - bb0, int(offB[b + 1]) - bb0)]
                nA = a1 - a0
                for i, (kind, c) in enumerate(mms):
                    st, sp = (i == 0), (i == len(mms) - 1)
                    if kind == "self":
                        nc.tensor.matmul(aggUT, u_own[:, b * 2:(b + 1) * 2],
                                         identb_t[:], start=st, stop=sp)
                    elif kind == "A":
                        nc.tensor.matmul(aggUT, ues[:, a0 + c, :], oh[:, c, :],
                                         start=st, stop=sp)
                    else:
                        nc.tensor.matmul(aggUT, ues[:, NCHA + bb0 + c, :],
                                         oh[:, nA + c, :], start=st, stop=sp)
                cU = wk.tile([2, 128], BF16, tag="cU")
                nc.scalar.copy(cU[:], aggUT)
                hpreT = tps.tile([H, 128], F32, tag="tp")
                nc.tensor.matmul(hpreT[:], W1b[:], cU[:], start=True, stop=True)
                e1 = wk.tile([128, 128], BF16, tag="e1")
                nc.vector.tensor_tensor(
                    e1[:], hpreT[:], dinvrep[:, b * 128:(b + 1) * 128], OP.mult)
                hT = wk.tile([128, 128], BF16, tag="hT")
                nc.scalar.activation(hT[:], e1[:], AF.Tanh, bias=bvec_t[:, 0:1])
                # prepare L2: t1 = (h @ W2) * dinv -> sbuild
                pp = prepps.tile([128, H], F32, tag="pp")
                nc.tensor.matmul(pp[:], hT[:], W2b[:], start=True, stop=True)
                nc.vector.tensor_scalar(
                    sbuild[:, b, :], pp[:], dinv_pm[:, b:b + 1], None, OP.mult)
                if b == NB2 - 1 and SKIP_AG not in (True, "first"):
                    half_ag(0)

        if SKIP_AG in (True, "first"):
            nc.sync.dma_start(
                shardA_int[0].rearrange("(b p) h -> p b h", p=128)[:, :, :],
                sbuild[:, 0:NB2, :],
            )
        else:
            half_ag(0, second=True)

        # ================= Layer 2 (transposed) + prepare L3 ================
        # pass 1: self + A-half chunks -> SBUF partial (overlaps the B half-AG)
        for (b0, b1, a0, a1, bb0, bb1) in groups:
            gA, _ = gather_group(tableA_int[0], None, H, a0, a1, bb0, bb0, GMAXA, GMAXB)
            oh = build_oh(a0, a1, bb0, bb0)
            for b in range(b0, b1):
                aggT = aggps.tile([128, 128], F32, tag="agg")
                mms = [("self", None)]
                mms += [("A", c) for c in range(int(offA[b]) - a0, int(offA[b + 1]) - a0)]
                for i, (kind, c) in enumerate(mms):
                    st, sp = (i == 0), (i == len(mms) - 1)
                    if kind == "self":
                        nc.tensor.matmul(aggT[:], sbuild[:, b, :], identb_t[:],
                                         start=st, stop=sp)
                    else:
                        nc.tensor.matmul(aggT[:], gA[:, c, :], oh[:, c, :],
                                         start=st, stop=sp)
                nc.scalar.copy(part[:, b, :], aggT[:])
        # pass 2: B-half chunks + partial -> epilogue + prepare L3
        for (b0, b1, a0, a1, bb0, bb1) in groups:
            _, gB = gather_group(None, tableB_int[0], H, a0, a0, bb0, bb1, GMAXA, GMAXB)
            oh = build_oh(a0, a0, bb0, bb1)
            for b in range(b0, b1):
                nB = int(offB[b + 1]) - int(offB[b])
                if nB:
                    aggT = aggps.tile([128, 128], F32, tag="agg")
                    for i, c in enumerate(range(int(offB[b]) - bb0, int(offB[b + 1]) - bb0)):
                        nc.tensor.matmul(aggT[:], gB[:, c, :], oh[:, c, :],
                                         start=(i == 0), stop=(i == nB - 1))
                    s1 = wk.tile([128, 128], F32, tag="s1")
                    nc.vector.tensor_tensor(s1[:], aggT[:], part[:, b, :], OP.add)
                    src_agg = s1
                else:
                    src_agg = part[:, b, :]
                e1 = wk.tile([128, 128], BF16, tag="e1")
                nc.vector.tensor_tensor(
                    e1[:], src_agg if nB == 0 else s1[:],
                    dinvrep[:, b * 128:(b + 1) * 128], OP.mult)
                hT = wk.tile([128, 128], BF16, tag="hT")
                nc.scalar.activation(hT[:], e1[:], AF.Tanh, bias=bvec_t[:, 1:2])
                pp = prepps.tile([128, H], F32, tag="pp")
                nc.tensor.matmul(pp[:], hT[:], W3b[:], start=True, stop=True)
                nc.vector.tensor_scalar(
                    sbuild[:, b, :], pp[:], dinv_pm[:, b:b + 1], None, OP.mult)
                if b == NB2 - 1 and SKIP_AG not in (True, "second"):
                    half_ag(1)

        if SKIP_AG in (True, "second"):
            nc.sync.dma_start(
                shardA_int[1].rearrange("(b p) h -> p b h", p=128)[:, :, :],
                sbuild[:, 0:NB2, :],
            )
        else:
            half_ag(1, second=True)

        # ================= Layer 3 (node-major) + pooling ===================
        # pass 1: self + A-half chunks -> SBUF partial (overlaps the B half-AG)
        for (b0, b1, a0, a1, bb0, bb1) in groups:
            gA, _ = gather_group(tableA_int[1], None, H, a0, a1, bb0, bb0, GMAXA, GMAXB)
            oh = build_oh(a0, a1, bb0, bb0)
            for b in range(b0, b1):
                agg = aggps.tile([128, H], F32, tag="agg")
                mms = [("self", None)]
                mms += [("A", c) for c in range(int(offA[b]) - a0, int(offA[b + 1]) - a0)]
                for i, (kind, c) in enumerate(mms):
                    st, sp = (i == 0), (i == len(mms) - 1)
                    if kind == "self":
                        nc.tensor.matmul(agg[:], identb_t[:], sbuild[:, b, :],
                                         start=st, stop=sp)
                    else:
                        nc.tensor.matmul(agg[:], oh[:, c, :], gA[:, c, :],
                                         start=st, stop=sp)
                nc.scalar.copy(part[:, b, :], agg[:])
        # pass 2: B-half chunks + partial -> epilogue + pooling
        for (b0, b1, a0, a1, bb0, bb1) in groups:
            _, gB = gather_group(None, tableB_int[1], H, a0, a0, bb0, bb1, GMAXA, GMAXB)
            oh = build_oh(a0, a0, bb0, bb1)
            for b in range(b0, b1):
                nB = int(offB[b + 1]) - int(offB[b])
                if nB:
                    agg = aggps.tile([128, H], F32, tag="agg")
                    for i, c in enumerate(range(int(offB[b]) - bb0, int(offB[b + 1]) - bb0)):
                        nc.tensor.matmul(agg[:], oh[:, c, :], gB[:, c, :],
                                         start=(i == 0), stop=(i == nB - 1))
                    s1 = wk.tile([128, H], F32, tag="s1")
                    nc.vector.tensor_tensor(s1[:], agg[:], part[:, b, :], OP.add)
                    src3 = s1[:]
                else:
                    src3 = part[:, b, :]
                e2 = wk.tile([128, H], F32, tag="e2")
                nc.vector.scalar_tensor_tensor(
                    e2[:], src3, dinv_pm[:, b:b + 1], brep3_t[:], OP.mult, OP.add)
                hblk = wk.tile([128, H], F32, tag="hblk")
                nc.scalar.activation(hblk[:], e2[:], AF.Tanh)
                # mean pool accumulate; max pool via PE transpose + free reduce
                nc.tensor.matmul(meanp[:], hblk[:], goh_t[:, b, :],
                                 start=(b == 0), stop=(b == NBLK - 1))
                tp = tps.tile([128, H], F32, tag="tp")
                nc.tensor.transpose(tp[:], hblk[:], ident_t[:])
                nc.vector.tensor_reduce(
                    pmax[:, b:b + 1], tp[:], mybir.AxisListType.X, OP.max)

        # ---- pooling tail + head ----
        p2 = resid.tile([128, NBLK], F32, tag="p2")
        nc.vector.tensor_scalar(p2[:], pmax[:], 2.0, None, OP.add)
        mg = wk.tile([128, GPC, NBLK], F32, tag="mg")
        nc.vector.tensor_tensor(
            mg[:], p2[:, None, :].broadcast_to((128, GPC, NBLK)),
            gmask_t[:].rearrange("p (g b) -> p g b", g=GPC), OP.mult)
        mxT = resid.tile([128, GPC], F32, tag="mxT")
        nc.vector.tensor_reduce(
            mxT[:, :, None], mg[:], mybir.AxisListType.X, OP.max)
        mxT2 = resid.tile([128, GPC], F32, tag="mxT2")
        nc.vector.scalar_tensor_tensor(
            mxT2[:], mxT[:], -2.0, gvalid_t[:], OP.add, OP.mult)

        cmax = wk.tile([128, GPC], F32, tag="cmax")
        nc.vector.tensor_scalar(cmax[:], cntrep_t[:], 1.0, None, OP.max)
        crec = wk.tile([128, GPC], F32, tag="crec")
        nc.vector.reciprocal(crec[:], cmax[:])
        meanT = wk.tile([128, GPC], F32, tag="meanT")
        nc.vector.tensor_tensor(meanT[:], meanp[:], crec[:], OP.mult)

        headp_full = prepps.tile([128, H], F32, tag="pp")
        headp = headp_full[0:GPC, 0:OC]
        nc.tensor.matmul(headp, mxT2[:], Wo_t[:, 0, :], start=True, stop=False)
        nc.tensor.matmul(headp, meanT[:], Wo_t[:, 1, :], start=False, stop=True)
        hsum = wk.tile([GPC, OC], F32, tag="hsum")
        nc.vector.tensor_tensor(hsum[:], headp, bo_t[:], OP.add)
        ofin = wk.tile([GPC, OC], F32, tag="ofin")
        nc.scalar.activation(ofin[:], hsum[:], AF.Tanh)
        nc.sync.dma_start(out_d[:], ofin[:])

    nc.compile()
    return nc


def make_in_maps(meta, inputs):
    colidx = np.tile(np.arange(128, dtype=np.float32), (128, 1)).astype(ml_dtypes.bfloat16)
    identb = np.eye(128, dtype=np.float32).astype(ml_dtypes.bfloat16)
    bvec = np.stack([np.asarray(inputs["b1"], np.float32),
                     np.asarray(inputs["b2"], np.float32)], axis=1)  # [128, 2]
    brep3 = np.tile(np.asarray(inputs["b3"], np.float32), (P, 1))
    bo_t = np.tile(np.asarray(inputs["bo"], np.float32), (GPC, 1))
    Wo = np.asarray(inputs["Wo"], np.float32)
    maps = []
    for c in meta["cores"]:
        maps.append({
            "idxA": c["idxA"], "idxB": c["idxB"],
            "dlA": c["dlA"], "dlB": c["dlB"],
            "colidx": colidx, "identb": identb, "ident": np.eye(128, dtype=np.float32),
            "W1": np.asarray(inputs["W1"], np.float32),
            "W2": np.asarray(inputs["W2"], np.float32),
            "W3": np.asarray(inputs["W3"], np.float32),
            "Wo": np.ascontiguousarray(np.stack([Wo[:H], Wo[H:]], axis=1)),
            "bo": bo_t, "bvec": bvec, "brep3": brep3,
            "gonehot": c["gonehot"], "gmask": c["gmask"], "gvalid": c["gvalid"],
            "cntrep": c["cntrep"],
            "deg_pm": c["deg_pm"], "degrow_rep": c["degrow_rep"],
            "deg_pm2_own": c["deg_pm2_own"], "xnm2_own": c["xnm2_own"],
            "xes": c["xes"], "deges": c["deges"],
        })
    return maps


_CACHE = {}


def kernel(x, edge_index, batch, W1, b1, W2, b2, W3, b3, Wo, bo):
    x = np.asarray(x, np.float32)
    edge_index = np.asarray(edge_index)
    batch = np.asarray(batch)
    meta = prep(x, edge_index, batch, 64)
    key = (meta["NBLK"], meta["NCHA"], meta["NCHB"])
    if key not in _CACHE:
        _CACHE[key] = build(meta)
    nc = _CACHE[key]
    inputs = dict(W1=W1, b1=b1, W2=W2, b2=b2, W3=W3, b3=b3, Wo=Wo, bo=bo)
    in_maps = make_in_maps(meta, inputs)
    res = run_bass_kernel_spmd(nc, in_maps, core_ids=list(range(8)), trace=False)
    out = np.concatenate([res.results[k]["out"] for k in range(8)], 0)
    return np.ascontiguousarray(out, dtype=np.float32)
